# revision 4
# baseline (speedup 1.0000x reference)
"""Trainium2 Bass kernel v2 for nn_DenoisingDiffusion_17025250361520.

Changes vs baseline (191.5us traced):
- The two h1/h2 AllGathers are gone: every core computes the full GCN
  (h1, h2 for all 1024 nodes) from the full host-prescaled adjacency
  A_hat = D^-1/2 (A_noisy + I) D^-1/2 (bf16, symmetric -> its natural
  row-major blocks are directly the matmul stationary operands, no
  transposes).  The GCN runs as 8 accumulating matmuls per layer with
  N=1024 bf16 moving operands, producing h^T layouts directly.
- A tiny dummy AllGather issues first, absorbing the ~42us first-
  collective rank-sync barrier behind local compute.
- The sigma <-> sigma^T exchange is one AllToAll split into 4 x 32-row
  bf16 chunks pipelined against the edge-MLP hot loop; only the last
  chunk's latency is exposed.
- Hot-loop relu tiles are produced by vector + scalar + gpsimd engines
  in parallel (the trace shows the PE matvec consumes a row in ~215ns
  while production took ~430ns with 2 producers).
- BCE tail in bf16.

Per-core inputs: the core's column block of A_hat (aloc), its adjacency
rows as uint8.  Everything else replicated.  Host sums the 8 partial
BCE sums.
"""

import numpy as np

N = 1024
NODE_DIM = 11
HIDDEN = 128
TIMESTEPS = 100
BETA_START, BETA_END = 1e-4, 0.02
NCORES = 8
R = N // NCORES  # 128 rows per core
CH = 32          # hot-loop chunk (rows per AllToAll)
NCH = R // CH
DEBUG = False

_CACHE = {}


# ----------------------------------------------------------------- host prep
def _parity_mask(t: int) -> np.ndarray:
    """Parity of the q_sample flip masks for steps 0..t (diag forced to 1 so
    |adj - P| directly includes the +I self loop)."""
    import jax
    import jax.numpy as jnp

    cpu = jax.devices("cpu")[0]
    with jax.default_device(cpu):
        betas = jnp.linspace(BETA_START, BETA_END, TIMESTEPS, dtype=jnp.float32)
        keys = jax.random.split(jax.random.key(42), t + 1)

        def step(c, kb):
            k, b = kb
            m = jax.random.uniform(k, (N, N)) < b
            return jnp.logical_xor(c, m), None

        par, _ = jax.lax.scan(
            step, jnp.zeros((N, N), bool), (keys, betas[: t + 1])
        )
        par = np.asarray(jax.device_get(par))
    p = np.triu(par, 1).astype(np.float32)
    p = p + p.T
    np.fill_diagonal(p, 1.0)
    return p


# ------------------------------------------------------------- device program
def _build_program():
    import concourse.bass as bass
    import concourse.mybir as mybir
    import concourse.tile as tile
    from concourse import bacc
    from concourse.bass import ts

    f32 = mybir.dt.float32
    bf16 = mybir.dt.bfloat16
    u8 = mybir.dt.uint8
    AL = mybir.AluOpType
    AF = mybir.ActivationFunctionType
    RG = [list(range(NCORES))]

    nc = bacc.Bacc(
        "TRN2", target_bir_lowering=False, debug=False, num_devices=NCORES
    )

    def din(name, shape, dt=f32):
        return nc.dram_tensor(name, shape, dt, kind="ExternalInput").ap()

    ablk_i = din("ablk", [128, NCORES, N], bf16)    # A_hat rows (j=b*128+p)
    aloc_i = din("aloc", [128, NCORES, 128], bf16)  # A_hat[:, local cols]
    xw1_i = din("xw1b", [128, NCORES, HIDDEN], bf16)
    w2_i = din("w2b", [HIDDEN, HIDDEN], bf16)
    wi_i = din("wib", [HIDDEN, HIDDEN], bf16)
    wj_i = din("wjb", [HIDDEN, HIDDEN], bf16)
    wv_i = din("wvb", [HIDDEN, 1], bf16)
    base_i = din("basec", [HIDDEN, 1])
    b2c_i = din("b2c", [HIDDEN, 1])
    id_i = din("idb", [128, 128], bf16)
    ones_i = din("onescol", [128, 1])
    zero_i = din("zerocol", [128, 1])
    adj_i = din("adj_u8", [R, N], u8)
    out_ap = nc.dram_tensor("out", [1, 1], f32, kind="ExternalOutput").ap()
    dbg_ap = (
        nc.dram_tensor("dbg", [128, 16], f32, kind="ExternalOutput").ap()
        if DEBUG
        else None
    )

    with tile.TileContext(nc) as tc:
        with (
            tc.tile_pool(name="const", bufs=1) as cp,
            tc.tile_pool(name="work", bufs=2) as wp,
            tc.tile_pool(name="hot", bufs=12) as hp,
            tc.tile_pool(name="ps", bufs=2, space="PSUM") as pp,
            tc.tile_pool(name="pbig", bufs=1, space="PSUM") as pb,
            tc.tile_pool(name="dram", bufs=1, space="DRAM") as dp,
        ):
            # ---- dummy collective: absorb the first-collective barrier.
            # Tiny AllToAll (cheaper than AllGather), fed by a dram->dram DMA
            # from an input tensor (content irrelevant) so it triggers at the
            # earliest possible point.
            # the dummy's payload is irrelevant: an unwritten internal DRAM
            # tile (stale bytes) keeps the feed DMA off the doorbell path
            dmy_b = dp.tile([NCORES, 4], bf16)
            dmy_all = dp.tile([NCORES, 4], bf16)
            nc.gpsimd.collective_compute(
                "AllToAll", AL.bypass, replica_groups=RG,
                ins=[dmy_b.opt()], outs=[dmy_all.opt()],
            )
            # consume the dummy here (stream-idle window, gpsimd is blocked
            # on the dummy anyway) so gpsimd joins the epilogue immediately
            # after the real exchange trigger
            dmys = wp.tile([1, 4], bf16)
            nc.gpsimd.dma_start(dmys, dmy_all[0:1, :])

            # ---- input DMAs (big ones split across queues)
            XW1 = cp.tile([128, NCORES, HIDDEN], bf16)
            nc.scalar.dma_start(XW1, xw1_i)
            ABLK = cp.tile([128, NCORES, N], bf16)
            nc.sync.dma_start(ABLK[:, 0:2, :], ablk_i[:, 0:2, :])
            nc.scalar.dma_start(ABLK[:, 2:4, :], ablk_i[:, 2:4, :])
            nc.scalar.dma_start(ABLK[:, 4:6, :], ablk_i[:, 4:6, :])
            nc.sync.dma_start(ABLK[:, 6:8, :], ablk_i[:, 6:8, :])
            ALOC = cp.tile([128, NCORES, 128], bf16)
            nc.sync.dma_start(ALOC, aloc_i)
            W2B = cp.tile([128, 128], bf16)
            nc.scalar.dma_start(W2B, w2_i)
            WIB = cp.tile([128, 128], bf16)
            nc.scalar.dma_start(WIB, wi_i)
            WJB = cp.tile([128, 128], bf16)
            nc.scalar.dma_start(WJB, wj_i)
            WVB = cp.tile([128, 1], bf16)
            nc.sync.dma_start(WVB, wv_i)
            BASEC = cp.tile([128, 1], f32)
            nc.sync.dma_start(BASEC, base_i)
            B2C = cp.tile([128, 1], f32)
            nc.sync.dma_start(B2C, b2c_i)
            IDB = cp.tile([128, 128], bf16)
            nc.sync.dma_start(IDB, id_i)
            ONESC = cp.tile([128, 1], f32)
            nc.sync.dma_start(ONESC, ones_i)
            ZEROC = cp.tile([128, 1], f32)
            nc.sync.dma_start(ZEROC, zero_i)
            ADJ8 = cp.tile([R, N], u8)
            nc.sync.dma_start(ADJ8, adj_i)

            # preload the sigmoid table set (relu is filler in every set, so
            # the hot loop then never switches sets; only Ln at the end does)
            SIGW = wp.tile([128, 1], f32)
            nc.scalar.activation(SIGW, B2C, AF.Sigmoid)

            # ---- GCN layer 1: H1T[h, j] = relu(sum_jb xw1[jb].T @ A[jb])
            # (matmul outputs split in halves: one PSUM bank caps N at 512 fp32)
            PH1 = pb.tile([128, N], f32, tag="big1")
            for jb in range(NCORES):
                for h in range(2):
                    nc.tensor.matmul(
                        PH1[:, ts(h, 512)], XW1[:, jb, :], ABLK[:, jb, ts(h, 512)],
                        start=(jb == 0), stop=(jb == NCORES - 1),
                    )
            H1T = cp.tile([128, N], bf16)
            nc.vector.tensor_scalar(H1T[:, 0:512], PH1[:, 0:512], 0.0, None, AL.max)
            nc.scalar.activation(H1T[:, 512:1024], PH1[:, 512:1024], AF.Relu)

            # ---- interlayer: M2[j, h'] = h1[j, :] @ w2  (block jb at a time)
            M2S = cp.tile([128, NCORES, 128], bf16)
            for jb in range(NCORES):
                pm = pp.tile([128, 128], f32, tag="sm")
                nc.tensor.matmul(pm, H1T[:, ts(jb, 128)], W2B, start=True, stop=True)
                if jb % 2 == 0:
                    nc.vector.tensor_copy(M2S[:, jb, :], pm)
                else:
                    nc.scalar.copy(M2S[:, jb, :], pm)

            # ---- GCN layer 2 (full, transposed) + local column block
            PH2 = pb.tile([128, N], f32, tag="big2")
            for jb in range(NCORES):
                for h in range(2):
                    nc.tensor.matmul(
                        PH2[:, ts(h, 512)], M2S[:, jb, :], ABLK[:, jb, ts(h, 512)],
                        start=(jb == 0), stop=(jb == NCORES - 1),
                    )
            PL2 = pp.tile([128, 128], f32, tag="sm")
            for jb in range(NCORES):
                nc.tensor.matmul(
                    PL2, M2S[:, jb, :], ALOC[:, jb, :],
                    start=(jb == 0), stop=(jb == NCORES - 1),
                )
            H2T = cp.tile([128, N], bf16)
            nc.vector.tensor_scalar(H2T[:, 0:512], PH2[:, 0:512], 0.0, None, AL.max)
            nc.scalar.activation(H2T[:, 512:1024], PH2[:, 512:1024], AF.Relu)
            H2LT = wp.tile([128, 128], bf16)
            nc.vector.tensor_scalar(H2LT, PL2, 0.0, None, AL.max)

            # ---- edge-MLP operands
            PJB = pb.tile([128, N], f32, tag="big1")
            for h in range(2):
                nc.tensor.matmul(
                    PJB[:, ts(h, 512)], WJB, H2T[:, ts(h, 512)],
                    start=True, stop=True,
                )
            HJB = cp.tile([128, N], bf16)
            nc.vector.tensor_copy(HJB[:, 0:512], PJB[:, 0:512])
            nc.scalar.copy(HJB[:, 512:1024], PJB[:, 512:1024])
            PIT = pp.tile([128, 128], f32, tag="sm")
            nc.tensor.matmul(PIT, WIB, H2LT, start=True, stop=True)
            HITf = cp.tile([128, 128], f32)
            nc.vector.tensor_scalar(HITf, PIT, BASEC, None, AL.add)

            # ---- hot loop: T = relu(HJB + HITf[:, i]) ; matvec over k
            PT0 = cp.tile([128, NCORES, R], bf16)   # sigma, [j%128, jb, i]
            TPSA = cp.tile([128, NCORES, R], bf16)  # sigma^T, [i, cblk, q]
            # sigma exchange via shared-HBM AllGather: each core dumps its
            # sigma rows to local DRAM; the shared-output AllGather then only
            # writes each core's 256KB slice into the common scratchpad (the 8
            # cores share one HBM), vs the ~24us software AllToAll path.
            sig_loc = nc.dram_tensor(
                "sig_loc", [128, NCORES, R], bf16, kind="Internal"
            )
            sig_sh = nc.dram_tensor(
                "sig_sh", [NCORES, 128, NCORES, R], bf16,
                kind="Internal", addr_space="Shared",
            )
            # last chunk is tiny so the post-loop sigmoid+staging exposure on
            # the exchange critical path is ~1us instead of ~4us
            lo = 0
            for csz in (32, 32, 32, 24, 8):
                LTP = pp.tile([128, NCORES, csz], f32, tag="lt")
                for q in range(csz):
                    i = lo + q
                    # DVE ~480ns/row vs ACT ~1134ns/row -> ~5:2 interleave.
                    # Separate tile tags per producer: sharing one rotation
                    # chains the fast producer behind the slow one via WAW
                    # buffer reuse.
                    if i % 7 in (2, 5):
                        T = hp.tile([128, N], bf16, tag="TA", bufs=4)
                        nc.scalar.activation(
                            T, HJB, AF.Relu, bias=HITf[:, i : i + 1]
                        )
                    else:
                        T = hp.tile([128, N], bf16, tag="TD", bufs=8)
                        nc.vector.tensor_scalar(
                            T, HJB, HITf[:, i : i + 1], 0.0, AL.add, AL.max
                        )
                    for jb in range(NCORES):
                        nc.tensor.matmul(
                            LTP[:, jb, q : q + 1], T[:, ts(jb, 128)], WVB,
                            start=True, stop=True,
                        )
                hi = lo + csz
                nc.scalar.activation(PT0[:, :, lo:hi], LTP, AF.Sigmoid, bias=B2C)
                # stage this chunk into the local exchange buffer (straight
                # dump, no rearrange); hidden under the next chunk's compute
                nc.sync.dma_start(
                    sig_loc.ap()[:, :, lo:hi],
                    PT0[:, :, lo:hi],
                )
                lo = hi
            # sigma exchange: shared-output AllGather, then a rank-offset
            # (dynamic) strided DMA pulls this core's column block back
            nc.gpsimd.collective_compute(
                "AllGather", AL.bypass, replica_groups=RG,
                ins=[sig_loc.ap().opt()], outs=[sig_sh.ap().opt()],
            )
            # preload the Ln table set during the collective wait (ACT is
            # idle then; without this the load serializes after the exchange
            # returns). Input is a last-chunk sigma value so the scheduler
            # cannot float this into the hot loop (a mid-loop set switch
            # costs ~2.6us).
            LNW = wp.tile([128, 1], f32)
            nc.scalar.activation(LNW, PT0[:, 0, R - 1 : R], AF.Ln)
            # return path: TPSA[p, s, q] = sig_sh[s, p, c_self, q].  The
            # c_self offset is the core's rank -> dynamic DMA with a
            # partition-id register offset.  Split across two queues; neither
            # on scalar, so ACT's Ln table load isn't blocked.
            SIGF = sig_sh.ap().rearrange("s p c q -> s p (c q)")
            pid_sy = nc.sync.partition_id()
            pid_gp = nc.gpsimd.partition_id()
            for s in range(NCORES):
                if s % 2 == 0:
                    nc.sync.dma_start(TPSA[:, s, :], SIGF[s, :, ts(pid_sy, R)])
                else:
                    nc.gpsimd.dma_start(TPSA[:, s, :], SIGF[s, :, ts(pid_gp, R)])

            # ---- p + p^T and BCE partial, pipelined in column halves
            # (ACT's Ln on half h overlaps DVE's chain on half h+1)
            PSB = pb.tile([128, NCORES, 128], bf16, tag="big2")
            for s in range(NCORES):
                nc.tensor.transpose(PSB[:, s, :], PT0[:, s, :], IDB)
            T2F = TPSA.rearrange("p c q -> p (c q)")
            P2F = PSB.rearrange("p s q -> p (s q)")
            AD = cp.tile([R, N], bf16)
            Q = wp.tile([R, N], bf16, bufs=1)
            PHT = wp.tile([R, N], bf16, bufs=1)
            LNQ = wp.tile([R, N], bf16, bufs=1)
            rs0 = wp.tile([R, 1], f32)
            rs1 = wp.tile([R, 1], f32)
            for h, rsh in ((0, rs0), (1, rs1)):
                sl = slice(h * 512, (h + 1) * 512)
                nc.vector.tensor_tensor(AD[:, sl], T2F[:, sl], P2F[:, sl], AL.add)
                # q = adj ? p_hat + eps : 1 - p_hat + eps   (p_hat = AD/2)
                nc.vector.tensor_scalar(
                    Q[:, sl], AD[:, sl], -0.5, 1.0 + 1e-12, AL.mult, AL.add
                )
                nc.vector.tensor_scalar(
                    PHT[:, sl], AD[:, sl], 0.5, 1e-12, AL.mult, AL.add
                )
                nc.vector.copy_predicated(Q[:, sl], ADJ8[:, sl], PHT[:, sl])
                nc.scalar.activation(
                    LNQ[:, sl], Q[:, sl], AF.Ln, bias=ZEROC, accum_out=rsh
                )
            psc = pp.tile([1, 1], f32, tag="sm")
            nc.tensor.matmul(psc, rs0, ONESC, start=True, stop=False)
            nc.tensor.matmul(psc, rs1, ONESC, start=False, stop=True)
            res = wp.tile([1, 1], f32)
            nc.vector.tensor_copy(res, psc)
            nc.sync.dma_start(out_ap, res)

            if DEBUG:
                DBG = wp.tile([128, 16], f32)
                nc.vector.tensor_copy(DBG[:, 0:1], HITf[:, 0:1])
                nc.vector.tensor_copy(DBG[:, 1:2], HJB[:, 0:1])
                nc.vector.tensor_copy(DBG[:, 2:3], H1T[:, 0:1])
                nc.vector.tensor_copy(DBG[:, 3:4], H2T[:, 0:1])
                nc.vector.tensor_copy(DBG[:, 4:5], H2LT[:, 0:1])
                nc.vector.tensor_copy(DBG[:, 5:6], PT0[:, 0, 0:1])
                nc.vector.tensor_copy(DBG[:, 6:7], TPSA[:, 0, 0:1])
                nc.vector.tensor_copy(DBG[:, 7:8], AD[:, 0:1])
                nc.vector.tensor_copy(DBG[:, 8:9], Q[:, 0:1])
                nc.vector.tensor_copy(DBG[:, 9:10], PHT[:, 0:1])
                nc.vector.tensor_copy(DBG[:, 10:11], rs)
                nc.vector.tensor_copy(DBG[:, 11:12], LNQ[:, 0:1])
                nc.vector.tensor_copy(DBG[:, 12:13], M2S[:, 0, 0:1])
                nc.vector.tensor_copy(DBG[:, 13:14], PT0[:, 7, 120:121])
                nc.vector.tensor_copy(DBG[:, 14:15], TPSA[:, 7, 120:121])
                nc.vector.tensor_copy(DBG[:, 15:16], ADJ8[:, 0:1])
                nc.sync.dma_start(dbg_ap, DBG)

    nc.compile()
    return nc


def _get_program():
    if "nc" not in _CACHE:
        _CACHE["nc"] = _build_program()
    return _CACHE["nc"]


# ------------------------------------------------------------------ interface
def make_in_maps(inputs):
    import ml_dtypes

    bf = ml_dtypes.bfloat16
    x = np.asarray(inputs["x"], np.float32)
    adj = np.asarray(inputs["adj"], np.float32)
    t = int(inputs["t"])
    w1 = np.asarray(inputs["w1"], np.float32)
    w2 = np.asarray(inputs["w2"], np.float32)
    mlp1_w = np.asarray(inputs["mlp1_w"], np.float32)
    mlp1_b = np.asarray(inputs["mlp1_b"], np.float32)
    mlp2_w = np.asarray(inputs["mlp2_w"], np.float32)
    mlp2_b = np.asarray(inputs["mlp2_b"], np.float32)
    time_emb = np.asarray(inputs["time_emb"], np.float32)

    P = _parity_mask(t)
    noisy = np.abs(adj - P)  # diag=1 in P -> includes +I
    dinv = (1.0 / np.sqrt(noisy.sum(axis=1, dtype=np.float32))).astype(np.float32)
    ahat = (noisy * dinv[:, None] * dinv[None, :]).astype(bf)
    ablk = np.ascontiguousarray(
        ahat.reshape(NCORES, 128, N).transpose(1, 0, 2)
    )  # [p, b, j] = ahat[b*128+p, :]
    xw1 = (x @ w1).astype(bf)
    xw1b = np.ascontiguousarray(xw1.reshape(NCORES, 128, HIDDEN).transpose(1, 0, 2))

    H = HIDDEN
    wi = np.ascontiguousarray(mlp1_w[:H]).astype(bf)
    wj = np.ascontiguousarray(mlp1_w[H : 2 * H]).astype(bf)
    w_t = mlp1_w[2 * H :]
    base = (time_emb[t] @ w_t + mlp1_b).astype(np.float32).reshape(H, 1)
    wv = np.ascontiguousarray(mlp2_w.reshape(H, 1)).astype(bf)
    b2c = np.full((H, 1), float(mlp2_b[0]), np.float32)
    idb = np.eye(128, dtype=np.float32).astype(bf)
    onescol = np.ones((128, 1), np.float32)
    zerocol = np.zeros((128, 1), np.float32)

    shared = {
        "ablk": ablk, "xw1b": xw1b, "w2b": w2.astype(bf), "wib": wi,
        "wjb": wj, "wvb": wv, "basec": base, "b2c": b2c, "idb": idb,
        "onescol": onescol, "zerocol": zerocol,
    }
    in_maps = []
    for c in range(NCORES):
        cols = slice(c * 128, (c + 1) * 128)
        aloc = np.ascontiguousarray(
            ahat[:, cols].reshape(NCORES, 128, 128).transpose(1, 0, 2)
        )
        in_maps.append(
            {
                "aloc": aloc,
                "adj_u8": np.ascontiguousarray(
                    adj[c * R : (c + 1) * R].astype(np.uint8)
                ),
                **shared,
            }
        )
    return in_maps


def run_device(in_maps, **kw):
    from concourse.bass_utils import run_bass_kernel_spmd

    nc = _get_program()
    return run_bass_kernel_spmd(nc, in_maps, list(range(NCORES)), **kw)


def kernel(**inputs) -> np.ndarray:
    in_maps = make_in_maps(inputs)
    res = run_device(in_maps)
    total = sum(float(res.results[c]["out"][0, 0]) for c in range(NCORES))
    loss = -total / float(N * N)
    return np.float32(loss)



# revision 10
# speedup vs baseline: 1.0126x; 1.0126x over previous
"""Trainium2 Bass kernel v2 for nn_DenoisingDiffusion_17025250361520.

Changes vs baseline (191.5us traced):
- The two h1/h2 AllGathers are gone: every core computes the full GCN
  (h1, h2 for all 1024 nodes) from the full host-prescaled adjacency
  A_hat = D^-1/2 (A_noisy + I) D^-1/2 (bf16, symmetric -> its natural
  row-major blocks are directly the matmul stationary operands, no
  transposes).  The GCN runs as 8 accumulating matmuls per layer with
  N=1024 bf16 moving operands, producing h^T layouts directly.
- A tiny dummy AllGather issues first, absorbing the ~42us first-
  collective rank-sync barrier behind local compute.
- The sigma <-> sigma^T exchange is one AllToAll split into 4 x 32-row
  bf16 chunks pipelined against the edge-MLP hot loop; only the last
  chunk's latency is exposed.
- Hot-loop relu tiles are produced by vector + scalar + gpsimd engines
  in parallel (the trace shows the PE matvec consumes a row in ~215ns
  while production took ~430ns with 2 producers).
- BCE tail in bf16.

Per-core inputs: the core's column block of A_hat (aloc), its adjacency
rows as uint8.  Everything else replicated.  Host sums the 8 partial
BCE sums.
"""

import numpy as np

N = 1024
NODE_DIM = 11
HIDDEN = 128
TIMESTEPS = 100
BETA_START, BETA_END = 1e-4, 0.02
NCORES = 8
R = N // NCORES  # 128 rows per core
CH = 32          # hot-loop chunk (rows per AllToAll)
NCH = R // CH
DEBUG = False

_CACHE = {}


# ----------------------------------------------------------------- host prep
def _parity_mask(t: int) -> np.ndarray:
    """Parity of the q_sample flip masks for steps 0..t (diag forced to 1 so
    |adj - P| directly includes the +I self loop)."""
    import jax
    import jax.numpy as jnp

    cpu = jax.devices("cpu")[0]
    with jax.default_device(cpu):
        betas = jnp.linspace(BETA_START, BETA_END, TIMESTEPS, dtype=jnp.float32)
        keys = jax.random.split(jax.random.key(42), t + 1)

        def step(c, kb):
            k, b = kb
            m = jax.random.uniform(k, (N, N)) < b
            return jnp.logical_xor(c, m), None

        par, _ = jax.lax.scan(
            step, jnp.zeros((N, N), bool), (keys, betas[: t + 1])
        )
        par = np.asarray(jax.device_get(par))
    p = np.triu(par, 1).astype(np.float32)
    p = p + p.T
    np.fill_diagonal(p, 1.0)
    return p


# ------------------------------------------------------------- device program
def _build_program():
    import concourse.bass as bass
    import concourse.mybir as mybir
    import concourse.tile as tile
    from concourse import bacc
    from concourse.bass import ts

    f32 = mybir.dt.float32
    bf16 = mybir.dt.bfloat16
    u8 = mybir.dt.uint8
    AL = mybir.AluOpType
    AF = mybir.ActivationFunctionType
    RG = [list(range(NCORES))]

    nc = bacc.Bacc(
        "TRN2", target_bir_lowering=False, debug=False, num_devices=NCORES
    )

    def din(name, shape, dt=f32):
        return nc.dram_tensor(name, shape, dt, kind="ExternalInput").ap()

    ablk_i = din("ablk", [128, NCORES, N], bf16)    # A_hat rows (j=b*128+p)
    aloc_i = din("aloc", [128, NCORES, 128], bf16)  # A_hat[:, local cols]
    xw1_i = din("xw1b", [128, NCORES, HIDDEN], bf16)
    w2_i = din("w2b", [HIDDEN, HIDDEN], bf16)
    wi_i = din("wib", [HIDDEN, HIDDEN], bf16)
    wj_i = din("wjb", [HIDDEN, HIDDEN], bf16)
    wv_i = din("wvb", [HIDDEN, 1], bf16)
    base_i = din("basec", [HIDDEN, 1])
    b2c_i = din("b2c", [HIDDEN, 1])
    id_i = din("idb", [128, 128], bf16)
    ones_i = din("onescol", [128, 1])
    zero_i = din("zerocol", [128, 1])
    adj_i = din("adj_u8", [R, N], u8)
    out_ap = nc.dram_tensor("out", [1, 1], f32, kind="ExternalOutput").ap()
    dbg_ap = (
        nc.dram_tensor("dbg", [128, 16], f32, kind="ExternalOutput").ap()
        if DEBUG
        else None
    )

    with tile.TileContext(nc) as tc:
        with (
            tc.tile_pool(name="const", bufs=1) as cp,
            tc.tile_pool(name="work", bufs=2) as wp,
            tc.tile_pool(name="hot", bufs=12) as hp,
            tc.tile_pool(name="ps", bufs=2, space="PSUM") as pp,
            tc.tile_pool(name="pbig", bufs=1, space="PSUM") as pb,
            tc.tile_pool(name="dram", bufs=1, space="DRAM") as dp,
        ):
            # ---- dummy collective: absorb the first-collective barrier.
            # Tiny AllToAll (cheaper than AllGather), fed by a dram->dram DMA
            # from an input tensor (content irrelevant) so it triggers at the
            # earliest possible point.
            # the dummy's payload is irrelevant: an unwritten internal DRAM
            # tile (stale bytes) keeps the feed DMA off the doorbell path
            dmy_b = dp.tile([NCORES, 4], bf16)
            dmy_all = dp.tile([NCORES, 4], bf16)
            nc.gpsimd.collective_compute(
                "AllToAll", AL.bypass, replica_groups=RG,
                ins=[dmy_b.opt()], outs=[dmy_all.opt()],
            )
            # consume the dummy here (stream-idle window, gpsimd is blocked
            # on the dummy anyway) so gpsimd joins the epilogue immediately
            # after the real exchange trigger
            dmys = wp.tile([1, 4], bf16)
            nc.gpsimd.dma_start(dmys, dmy_all[0:1, :])

            # ---- input DMAs (big ones split across queues)
            XW1 = cp.tile([128, NCORES, HIDDEN], bf16)
            nc.scalar.dma_start(XW1, xw1_i)
            ABLK = cp.tile([128, NCORES, N], bf16)
            nc.sync.dma_start(ABLK[:, 0:2, :], ablk_i[:, 0:2, :])
            nc.scalar.dma_start(ABLK[:, 2:4, :], ablk_i[:, 2:4, :])
            nc.scalar.dma_start(ABLK[:, 4:6, :], ablk_i[:, 4:6, :])
            nc.sync.dma_start(ABLK[:, 6:8, :], ablk_i[:, 6:8, :])
            ALOC = cp.tile([128, NCORES, 128], bf16)
            nc.sync.dma_start(ALOC, aloc_i)
            W2B = cp.tile([128, 128], bf16)
            nc.scalar.dma_start(W2B, w2_i)
            WIB = cp.tile([128, 128], bf16)
            nc.scalar.dma_start(WIB, wi_i)
            WJB = cp.tile([128, 128], bf16)
            nc.scalar.dma_start(WJB, wj_i)
            WVB = cp.tile([128, 1], bf16)
            nc.sync.dma_start(WVB, wv_i)
            BASEC = cp.tile([128, 1], f32)
            nc.sync.dma_start(BASEC, base_i)
            B2C = cp.tile([128, 1], f32)
            nc.sync.dma_start(B2C, b2c_i)
            IDB = cp.tile([128, 128], bf16)
            nc.sync.dma_start(IDB, id_i)
            ONESC = cp.tile([128, 1], f32)
            nc.sync.dma_start(ONESC, ones_i)
            ZEROC = cp.tile([128, 1], f32)
            nc.sync.dma_start(ZEROC, zero_i)
            ADJ8 = cp.tile([R, N], u8)
            nc.sync.dma_start(ADJ8, adj_i)

            # preload the sigmoid table set (relu is filler in every set, so
            # the hot loop then never switches sets; only Ln at the end does)
            SIGW = wp.tile([128, 1], f32)
            nc.scalar.activation(SIGW, B2C, AF.Sigmoid)

            # ---- GCN layer 1: H1T[h, j] = relu(sum_jb xw1[jb].T @ A[jb])
            # (matmul outputs split in halves: one PSUM bank caps N at 512 fp32)
            PH1 = pb.tile([128, N], f32, tag="big1")
            for jb in range(NCORES):
                for h in range(2):
                    nc.tensor.matmul(
                        PH1[:, ts(h, 512)], XW1[:, jb, :], ABLK[:, jb, ts(h, 512)],
                        start=(jb == 0), stop=(jb == NCORES - 1),
                    )
            H1T = cp.tile([128, N], bf16)
            nc.vector.tensor_scalar(H1T[:, 0:512], PH1[:, 0:512], 0.0, None, AL.max)
            nc.scalar.activation(H1T[:, 512:1024], PH1[:, 512:1024], AF.Relu)

            # ---- interlayer: M2[j, h'] = h1[j, :] @ w2  (block jb at a time)
            M2S = cp.tile([128, NCORES, 128], bf16)
            for jb in range(NCORES):
                pm = pp.tile([128, 128], f32, tag="sm")
                nc.tensor.matmul(pm, H1T[:, ts(jb, 128)], W2B, start=True, stop=True)
                if jb % 2 == 0:
                    nc.vector.tensor_copy(M2S[:, jb, :], pm)
                else:
                    nc.scalar.copy(M2S[:, jb, :], pm)

            # ---- GCN layer 2 (full, transposed) + local column block
            PH2 = pb.tile([128, N], f32, tag="big2")
            for jb in range(NCORES):
                for h in range(2):
                    nc.tensor.matmul(
                        PH2[:, ts(h, 512)], M2S[:, jb, :], ABLK[:, jb, ts(h, 512)],
                        start=(jb == 0), stop=(jb == NCORES - 1),
                    )
            PL2 = pp.tile([128, 128], f32, tag="sm")
            for jb in range(NCORES):
                nc.tensor.matmul(
                    PL2, M2S[:, jb, :], ALOC[:, jb, :],
                    start=(jb == 0), stop=(jb == NCORES - 1),
                )
            H2T = cp.tile([128, N], bf16)
            nc.vector.tensor_scalar(H2T[:, 0:512], PH2[:, 0:512], 0.0, None, AL.max)
            nc.scalar.activation(H2T[:, 512:1024], PH2[:, 512:1024], AF.Relu)
            H2LT = wp.tile([128, 128], bf16)
            nc.vector.tensor_scalar(H2LT, PL2, 0.0, None, AL.max)

            # ---- edge-MLP operands
            PJB = pb.tile([128, N], f32, tag="big1")
            for h in range(2):
                nc.tensor.matmul(
                    PJB[:, ts(h, 512)], WJB, H2T[:, ts(h, 512)],
                    start=True, stop=True,
                )
            HJB = cp.tile([128, N], bf16)
            nc.vector.tensor_copy(HJB[:, 0:512], PJB[:, 0:512])
            nc.scalar.copy(HJB[:, 512:1024], PJB[:, 512:1024])
            PIT = pp.tile([128, 128], f32, tag="sm")
            nc.tensor.matmul(PIT, WIB, H2LT, start=True, stop=True)
            HITf = cp.tile([128, 128], f32)
            nc.vector.tensor_scalar(HITf, PIT, BASEC, None, AL.add)

            # ---- hot loop: T = relu(HJB + HITf[:, i]) ; matvec over k
            PT0 = cp.tile([128, NCORES, R], bf16)   # sigma, [j%128, jb, i]
            TPSA = cp.tile([128, NCORES, R], bf16)  # sigma^T, [i, cblk, q]
            a_in = dp.tile([NCORES, R, R], bf16)
            a_out = dp.tile([NCORES, R, R], bf16)
            # last chunk is tiny so the post-loop sigmoid+staging exposure on
            # the exchange critical path is ~1us instead of ~4us
            lo = 0
            for csz in (32, 32, 32, 24, 8):
                LTP = pp.tile([128, NCORES, csz], f32, tag="lt")
                for q in range(csz):
                    i = lo + q
                    # DVE ~480ns/row vs ACT ~1134ns/row -> ~5:2 interleave.
                    # Separate tile tags per producer: sharing one rotation
                    # chains the fast producer behind the slow one via WAW
                    # buffer reuse.
                    if i % 7 in (2, 5):
                        T = hp.tile([128, N], bf16, tag="TA", bufs=4)
                        nc.scalar.activation(
                            T, HJB, AF.Relu, bias=HITf[:, i : i + 1]
                        )
                    else:
                        T = hp.tile([128, N], bf16, tag="TD", bufs=8)
                        nc.vector.tensor_scalar(
                            T, HJB, HITf[:, i : i + 1], 0.0, AL.add, AL.max
                        )
                    for jb in range(NCORES):
                        nc.tensor.matmul(
                            LTP[:, jb, q : q + 1], T[:, ts(jb, 128)], WVB,
                            start=True, stop=True,
                        )
                hi = lo + csz
                nc.scalar.activation(PT0[:, :, lo:hi], LTP, AF.Sigmoid, bias=B2C)
                # stage this chunk into the exchange buffer (dram-side
                # rearrange keeps the sbuf AP partition-major); hidden under
                # the next chunk's compute
                nc.sync.dma_start(
                    a_in.rearrange("s p q -> p s q")[:, :, lo:hi],
                    PT0[:, :, lo:hi],
                )
                lo = hi
            # single sigma exchange: one AllToAll, one strided DMA back
            # (the PE transposes overlap the A2A wait)
            nc.gpsimd.collective_compute(
                "AllToAll", AL.bypass, replica_groups=RG,
                ins=[a_in.opt()], outs=[a_out.opt()],
            )
            # ---- TEMPORARY probes: measure collective latencies ----
            RGP = [[2 * k, 2 * k + 1] for k in range(NCORES // 2)]
            pr1i = dp.tile([2, 4], bf16)
            pr1o = dp.tile([2, 4], bf16)
            nc.gpsimd.collective_compute(
                "AllReduce", AL.add, replica_groups=RGP,
                ins=[pr1i.opt()], outs=[pr1o.opt()],
            )
            pr1s = wp.tile([1, 4], bf16)
            nc.gpsimd.dma_start(pr1s, pr1o[0:1, :])
            pr2i = dp.tile([NCORES, 4], bf16)
            pr2o = dp.tile([NCORES, 4], bf16)
            nc.gpsimd.collective_compute(
                "AllToAll", AL.bypass, replica_groups=RG,
                ins=[pr2i.opt()], outs=[pr2o.opt()],
            )
            pr3i = dp.tile([NCORES, 4], bf16)
            pr3o = dp.tile([NCORES, 4], bf16)
            nc.gpsimd.collective_compute(
                "AllToAll", AL.bypass, replica_groups=RG,
                ins=[pr3i.opt()], outs=[pr3o.opt()],
            )
            pr2s = wp.tile([1, 4], bf16)
            nc.gpsimd.dma_start(pr2s, pr2o[0:1, :])
            pr3s = wp.tile([1, 4], bf16)
            nc.gpsimd.dma_start(pr3s, pr3o[0:1, :])
            # ---- end probes ----
            # preload the Ln table set during the A2A wait (ACT is idle then;
            # without this the load serializes after the exchange returns).
            # Input is a last-chunk sigma value so the scheduler cannot float
            # this into the hot loop (a mid-loop set switch costs ~2.6us).
            LNW = wp.tile([128, 1], f32)
            nc.scalar.activation(LNW, PT0[:, 0, R - 1 : R], AF.Ln)
            # return path split across two queues (halves align with j-cols);
            # neither on scalar, so ACT's table load isn't blocked
            # (scalar's Ln table preload was emitted above, so parking the
            # second half on the scalar queue can no longer delay it)
            nc.sync.dma_start(
                TPSA[:, 0:4, :], a_out[0:4].rearrange("s p q -> p s q")
            )
            nc.scalar.dma_start(
                TPSA[:, 4:8, :], a_out[4:8].rearrange("s p q -> p s q")
            )

            # ---- p + p^T and BCE partial, pipelined in column halves
            # (ACT's Ln on half h overlaps DVE's chain on half h+1)
            PSB = pb.tile([128, NCORES, 128], bf16, tag="big2")
            for s in range(NCORES):
                nc.tensor.transpose(PSB[:, s, :], PT0[:, s, :], IDB)
            T2F = TPSA.rearrange("p c q -> p (c q)")
            P2F = PSB.rearrange("p s q -> p (s q)")
            AD = cp.tile([R, N], bf16)
            Q = wp.tile([R, N], bf16, bufs=1)
            PHT = wp.tile([R, N], bf16, bufs=1)
            LNQ = wp.tile([R, N], bf16, bufs=1)
            rs0 = wp.tile([R, 1], f32)
            rs1 = wp.tile([R, 1], f32)
            for h, rsh in ((0, rs0), (1, rs1)):
                sl = slice(h * 512, (h + 1) * 512)
                nc.vector.tensor_tensor(AD[:, sl], T2F[:, sl], P2F[:, sl], AL.add)
                # q = adj ? p_hat + eps : 1 - p_hat + eps   (p_hat = AD/2)
                nc.vector.tensor_scalar(
                    Q[:, sl], AD[:, sl], -0.5, 1.0 + 1e-12, AL.mult, AL.add
                )
                nc.vector.tensor_scalar(
                    PHT[:, sl], AD[:, sl], 0.5, 1e-12, AL.mult, AL.add
                )
                nc.vector.copy_predicated(Q[:, sl], ADJ8[:, sl], PHT[:, sl])
                nc.scalar.activation(
                    LNQ[:, sl], Q[:, sl], AF.Ln, bias=ZEROC, accum_out=rsh
                )
            psc = pp.tile([1, 1], f32, tag="sm")
            nc.tensor.matmul(psc, rs0, ONESC, start=True, stop=False)
            nc.tensor.matmul(psc, rs1, ONESC, start=False, stop=True)
            res = wp.tile([1, 1], f32)
            nc.vector.tensor_copy(res, psc)
            nc.sync.dma_start(out_ap, res)

            if DEBUG:
                DBG = wp.tile([128, 16], f32)
                nc.vector.tensor_copy(DBG[:, 0:1], HITf[:, 0:1])
                nc.vector.tensor_copy(DBG[:, 1:2], HJB[:, 0:1])
                nc.vector.tensor_copy(DBG[:, 2:3], H1T[:, 0:1])
                nc.vector.tensor_copy(DBG[:, 3:4], H2T[:, 0:1])
                nc.vector.tensor_copy(DBG[:, 4:5], H2LT[:, 0:1])
                nc.vector.tensor_copy(DBG[:, 5:6], PT0[:, 0, 0:1])
                nc.vector.tensor_copy(DBG[:, 6:7], TPSA[:, 0, 0:1])
                nc.vector.tensor_copy(DBG[:, 7:8], AD[:, 0:1])
                nc.vector.tensor_copy(DBG[:, 8:9], Q[:, 0:1])
                nc.vector.tensor_copy(DBG[:, 9:10], PHT[:, 0:1])
                nc.vector.tensor_copy(DBG[:, 10:11], rs)
                nc.vector.tensor_copy(DBG[:, 11:12], LNQ[:, 0:1])
                nc.vector.tensor_copy(DBG[:, 12:13], M2S[:, 0, 0:1])
                nc.vector.tensor_copy(DBG[:, 13:14], PT0[:, 7, 120:121])
                nc.vector.tensor_copy(DBG[:, 14:15], TPSA[:, 7, 120:121])
                nc.vector.tensor_copy(DBG[:, 15:16], ADJ8[:, 0:1])
                nc.sync.dma_start(dbg_ap, DBG)

    nc.compile()
    return nc


def _get_program():
    if "nc" not in _CACHE:
        _CACHE["nc"] = _build_program()
    return _CACHE["nc"]


# ------------------------------------------------------------------ interface
def make_in_maps(inputs):
    import ml_dtypes

    bf = ml_dtypes.bfloat16
    x = np.asarray(inputs["x"], np.float32)
    adj = np.asarray(inputs["adj"], np.float32)
    t = int(inputs["t"])
    w1 = np.asarray(inputs["w1"], np.float32)
    w2 = np.asarray(inputs["w2"], np.float32)
    mlp1_w = np.asarray(inputs["mlp1_w"], np.float32)
    mlp1_b = np.asarray(inputs["mlp1_b"], np.float32)
    mlp2_w = np.asarray(inputs["mlp2_w"], np.float32)
    mlp2_b = np.asarray(inputs["mlp2_b"], np.float32)
    time_emb = np.asarray(inputs["time_emb"], np.float32)

    P = _parity_mask(t)
    noisy = np.abs(adj - P)  # diag=1 in P -> includes +I
    dinv = (1.0 / np.sqrt(noisy.sum(axis=1, dtype=np.float32))).astype(np.float32)
    ahat = (noisy * dinv[:, None] * dinv[None, :]).astype(bf)
    ablk = np.ascontiguousarray(
        ahat.reshape(NCORES, 128, N).transpose(1, 0, 2)
    )  # [p, b, j] = ahat[b*128+p, :]
    xw1 = (x @ w1).astype(bf)
    xw1b = np.ascontiguousarray(xw1.reshape(NCORES, 128, HIDDEN).transpose(1, 0, 2))

    H = HIDDEN
    wi = np.ascontiguousarray(mlp1_w[:H]).astype(bf)
    wj = np.ascontiguousarray(mlp1_w[H : 2 * H]).astype(bf)
    w_t = mlp1_w[2 * H :]
    base = (time_emb[t] @ w_t + mlp1_b).astype(np.float32).reshape(H, 1)
    wv = np.ascontiguousarray(mlp2_w.reshape(H, 1)).astype(bf)
    b2c = np.full((H, 1), float(mlp2_b[0]), np.float32)
    idb = np.eye(128, dtype=np.float32).astype(bf)
    onescol = np.ones((128, 1), np.float32)
    zerocol = np.zeros((128, 1), np.float32)

    shared = {
        "ablk": ablk, "xw1b": xw1b, "w2b": w2.astype(bf), "wib": wi,
        "wjb": wj, "wvb": wv, "basec": base, "b2c": b2c, "idb": idb,
        "onescol": onescol, "zerocol": zerocol,
    }
    in_maps = []
    for c in range(NCORES):
        cols = slice(c * 128, (c + 1) * 128)
        aloc = np.ascontiguousarray(
            ahat[:, cols].reshape(NCORES, 128, 128).transpose(1, 0, 2)
        )
        in_maps.append(
            {
                "aloc": aloc,
                "adj_u8": np.ascontiguousarray(
                    adj[c * R : (c + 1) * R].astype(np.uint8)
                ),
                **shared,
            }
        )
    return in_maps


def run_device(in_maps, **kw):
    from concourse.bass_utils import run_bass_kernel_spmd

    nc = _get_program()
    return run_bass_kernel_spmd(nc, in_maps, list(range(NCORES)), **kw)


def kernel(**inputs) -> np.ndarray:
    in_maps = make_in_maps(inputs)
    res = run_device(in_maps)
    total = sum(float(res.results[c]["out"][0, 0]) for c in range(NCORES))
    loss = -total / float(N * N)
    return np.float32(loss)

